# revision 1
# baseline (speedup 1.0000x reference)
"""MultiHeadAttention Trainium2 kernel (8 NeuronCores, SPMD).

Sharding: core c = (batch b=c//2, head-group g=c%2). Each core computes
8 of the 16 heads for one batch: Q/K/V projections restricted to the
512 d_model dims of its head group, full attention for those heads, and
a partial output projection. Host sums the two head-group partials per
batch and adds the output bias.

Layout (all bf16 matmuls, f32 accumulation in PSUM):
  x (q/k/v)  : DMAed ONCE into SBUF as 8 chunks of [128 d_in, 2048 tok]
  wq/wk/wv   : (1024, 512)  = W[rows g].T   (d_in, d_out_local)
  wo         : (512, 1024)  = Wo[:, cols g].T
  qhT/khT    : (128, 2048) x4 pairs  (head dims on partitions)
  vh         : (128 tok, 8*65) x16 token blocks; per head 64 v-dims
               plus a ones column so the ctx matmul also produces the
               softmax denominator in PSUM row 64.
  ctxT       : (128, 2048) x4 pairs

The TRN2 PE clock ramps 0.65 -> 1.2 -> 2.4 GHz and needs ~3us of
gap-free execution to reach full rate; every stall resets it. So the
kernel is organized to keep the PE stream dependency-free:
  - x is resident (projections never wait on DMA after the prefetch)
  - weight-stationary projection loops minimize LDWEIGHTS
  - scores run up to 3 PSUM tiles ahead of ctx, so exp latency is hidden
  - softmax normalization (broadcast + approx-reciprocal + mul) happens
    on DVE from an SBUF staging copy, off the PE critical path
  - the output projection splits each token block into a pairs-0..2
    PSUM group (emitted ahead, independent of the last normalize) and a
    pair-3 extension, so the PE never idles at the phase boundary

Measured (unthrottled): ~448-452us/core, of which ~25us DMA ramp-in,
~89us projections, ~281us attention (ACT/exp-engine saturated at its
~285us floor: 33.5M exps at 153.6G/s + 0.26us/instr overhead), ~54us
output projection + drain. The attention phase is exp-throughput-bound;
PE sits at ~94% of that pace, so further gains need a second exp path
(none exists on TRN2) or lower precision (fails the 2e-2 gate).
"""

import numpy as np
import ml_dtypes

BF16 = ml_dtypes.bfloat16

B, S, D, H = 4, 2048, 1024, 16
DH = 64          # head dim
DL = 512         # local d_out (8 heads)
P = 128          # partitions
NCORES = 8
SCALE = 1.0 / np.sqrt(DH)

_CACHE = {}
LAST_RESULTS = None  # stashed BassKernelResults for test harness


def _build_nc():
    import concourse.bass as bass
    from concourse import bacc, mybir
    from concourse.tile import TileContext

    f32 = mybir.dt.float32
    bf16 = mybir.dt.bfloat16

    nc = bacc.Bacc("TRN2", target_bir_lowering=False, debug=False, num_devices=NCORES)

    # x pre-transposed on host: (8 d_in chunks, 128, 2048 tokens)
    xq_d = nc.dram_tensor("xq", (8, P, S), bf16, kind="ExternalInput")
    xk_d = nc.dram_tensor("xk", (8, P, S), bf16, kind="ExternalInput")
    xv_d = nc.dram_tensor("xv", (8, P, S), bf16, kind="ExternalInput")
    # weights pre-tiled on host to partition-major so DMA lines are 4-8KB
    wq_d = nc.dram_tensor("wq", (P, 8, DL), bf16, kind="ExternalInput")
    wk_d = nc.dram_tensor("wk", (P, 8, DL), bf16, kind="ExternalInput")
    wv_d = nc.dram_tensor("wv", (P, 8, DL), bf16, kind="ExternalInput")
    wo_d = nc.dram_tensor("wo", (P, 4, D), bf16, kind="ExternalInput")
    bq_d = nc.dram_tensor("bq", (P, 4), f32, kind="ExternalInput")
    bk_d = nc.dram_tensor("bk", (P, 4), f32, kind="ExternalInput")
    bvb_d = nc.dram_tensor("bvb", (P, 520), bf16, kind="ExternalInput")
    out_d = nc.dram_tensor("out", (S, D), bf16, kind="ExternalOutput")

    Exp = mybir.ActivationFunctionType.Exp

    with TileContext(nc) as tc:
        with tc.tile_pool(name="res", bufs=1) as res:
            # ---------------- resident SBUF ----------------
            wq_sb = res.tile([P, 8, DL], bf16)
            wk_sb = res.tile([P, 8, DL], bf16)
            wv_sb = res.tile([P, 8, DL], bf16)
            wo_sb = res.tile([P, 4, D], bf16)
            bq_sb = res.tile([P, 4], f32)
            bk_sb = res.tile([P, 4], f32)
            bvb_sb = res.tile([P, 520], bf16)

            # two x buffers: A holds xq then xv, B holds xk
            xa = [res.tile([P, S], bf16, name=f"xa{c}") for c in range(8)]
            xb = [res.tile([P, S], bf16, name=f"xb{c}") for c in range(8)]

            qhT = [res.tile([P, S], bf16, name=f"qhT{i}") for i in range(4)]
            khT = [res.tile([P, S], bf16, name=f"khT{i}") for i in range(4)]
            vh = [res.tile([P, 8 * 65], bf16, name=f"vh{i}") for i in range(16)]
            ctxT = [res.tile([P, 1024], bf16, name=f"ctxT{i}") for i in range(8)]

            # ---------------- prefetch DMAs ----------------
            # wq leads the sync queue, two xq chunks lead gpsimd: everything
            # the first projection needs lands in parallel across 3 queues
            qs = [nc.sync, nc.scalar]
            nc.sync.dma_start(out=wq_sb, in_=wq_d.ap())
            nc.scalar.dma_start(out=bq_sb, in_=bq_d.ap())
            nc.gpsimd.dma_start(out=xa[6], in_=xq_d.ap()[6])
            nc.gpsimd.dma_start(out=xa[7], in_=xq_d.ap()[7])
            for c in range(6):
                qs[c % 2].dma_start(out=xa[c], in_=xq_d.ap()[c])
            nc.gpsimd.dma_start(out=wk_sb, in_=wk_d.ap())
            nc.gpsimd.dma_start(out=bk_sb, in_=bk_d.ap())
            nc.gpsimd.dma_start(out=wv_sb, in_=wv_d.ap())
            nc.gpsimd.dma_start(out=bvb_sb, in_=bvb_d.ap())
            nc.gpsimd.dma_start(out=wo_sb, in_=wo_d.ap())
            for c in range(8):
                qs[c % 2].dma_start(out=xb[c], in_=xk_d.ap()[c])

            # ---------------- phase 1: projections ----------------
            with tc.tile_pool(name="pj", bufs=8, space="PSUM") as pj:
                def proj_qk(x_sb, w_sb, b_sb, dst):
                    # weight-stationary: one LDW per (pair, c-chunk), the four
                    # token tiles stream against it
                    for p in range(4):
                        ps = [pj.tile([P, 512], f32, name="pjt", tag="pj")
                              for _ in range(4)]
                        for c in range(8):
                            for t in range(4):
                                nc.tensor.matmul(
                                    ps[t], lhsT=w_sb[:, c, p * P:(p + 1) * P],
                                    rhs=x_sb[c][:, t * 512:(t + 1) * 512],
                                    start=(c == 0), stop=(c == 7))
                        for t in range(4):
                            nc.vector.tensor_scalar_add(
                                out=dst[p][:, t * 512:(t + 1) * 512],
                                in0=ps[t], scalar1=b_sb[:, p:p + 1])

                proj_qk(xa, wq_sb, bq_sb, qhT)
                # xv reuses the xa tiles: emitted AFTER the Q projection so
                # each chunk's DMA waits (WAR) for Q-proj's reads, and V-proj
                # below waits (RAW) for the DMA. Overlaps the K projection.
                for c in range(8):
                    qs[c % 2].dma_start(out=xa[c], in_=xv_d.ap()[c])
                proj_qk(xb, wk_sb, bk_sb, khT)

                # V: x-stationary so the output lands token-partitioned
                for tb in range(16):
                    psv = pj.tile([P, 512], f32, name="psv", tag="pj")
                    for c in range(8):
                        nc.tensor.matmul(
                            psv, lhsT=xa[c][:, tb * P:(tb + 1) * P],
                            rhs=wv_sb[:, c, :],
                            start=(c == 0), stop=(c == 7))
                    vt = vh[tb].rearrange("p (h e) -> p h e", e=65)
                    nc.vector.tensor_copy(
                        vt[:, :, 0:64],
                        psv.rearrange("p (h e) -> p h e", e=64))
                    nc.gpsimd.memset(vt[:, :, 64:65], 1.0)
                    nc.vector.tensor_add(vh[tb], vh[tb], bvb_sb)

            # ---------------- phase 2: attention ----------------
            with tc.tile_pool(name="sc", bufs=3, space="PSUM") as sc, \
                 tc.tile_pool(name="cx", bufs=1, space="PSUM") as cx, \
                 tc.tile_pool(name="pt", bufs=6) as ptp, \
                 tc.tile_pool(name="st", bufs=2) as stp, \
                 tc.tile_pool(name="sm", bufs=2) as sm:

                def attn_chunk(p, hh, qh):
                    h = 2 * p + hh
                    po = 64 * hh
                    q0 = qh * 1024
                    cps = cx.tile([P, 1024], f32, name="cps", tag="cx")
                    for kb in range(16):
                        sps = sc.tile([P, 1024], f32, name="sps", tag="s")
                        for j in range(2):
                            nc.tensor.matmul(
                                sps[:, j * 512:(j + 1) * 512],
                                lhsT=khT[p][po:po + 64, kb * P:(kb + 1) * P],
                                rhs=qhT[p][po:po + 64, q0 + j * 512:q0 + (j + 1) * 512],
                                start=True, stop=True)
                        pt = ptp.tile([P, 1024], bf16, name="ptt", tag="pt")
                        nc.scalar.activation(pt, sps, Exp, scale=SCALE)
                        for j in range(2):
                            nc.tensor.matmul(
                                cps[0:65, j * 512:(j + 1) * 512],
                                lhsT=vh[kb][:, 65 * h:65 * h + 65],
                                rhs=pt[:, j * 512:(j + 1) * 512],
                                start=(kb == 0), stop=(kb == 15))
                    # stage ctx+denominator to SBUF, freeing the PSUM bank;
                    # normalize on DVE off the PE critical path
                    stg = stp.tile([P, 1024], f32, name="stg", tag="st")
                    nc.vector.tensor_copy(stg[0:65, :], cps[0:65, :])
                    rc = sm.tile([1, 1024], f32, name="rc", tag="rc")
                    nc.gpsimd.dma_start(out=rc, in_=stg[64:65, :])
                    step = (list(rc.ap[1])[0]
                            if hasattr(rc.ap[1], "__iter__") else 1)
                    bc = sm.tile([64, 1024], f32, name="bc", tag="bc")
                    nc.gpsimd.dma_start(
                        out=bc,
                        in_=bass.AP(tensor=rc.tensor, offset=rc.offset,
                                    ap=[[1, 1], [0, 64], [step, 1024]]))
                    rb = sm.tile([64, 1024], f32, name="rb", tag="rb")
                    nc.vector.reciprocal_approx_fast(rb, bc)
                    nc.vector.tensor_mul(
                        ctxT[2 * p + qh][po:po + 64, :], stg[0:64, :], rb)

                for p in range(4):
                    for hh in range(2):
                        for qh in range(2):
                            attn_chunk(p, hh, qh)

            # ---------------- phase 3: output projection ----------------
            # Split each token block's accumulation: pairs 0-2 (group A) don't
            # depend on the final attention chunk's normalize, so several A
            # groups are emitted ahead; pair 3 extends the same PSUM group
            # (start=False) once the last ctxT lands. The in-order PE queue
            # then has real work to chew while the normalize chain drains.
            with tc.tile_pool(name="po", bufs=4, space="PSUM") as pop, \
                 tc.tile_pool(name="ot", bufs=3) as otp:
                stq = [nc.sync, nc.scalar]
                tiles = {}

                def emit_a(qb):
                    oa = pop.tile([P, D], f32, name="ops", tag="po")
                    tiles[qb] = oa
                    for n in range(2):
                        for p in range(3):
                            nc.tensor.matmul(
                                oa[:, n * 512:(n + 1) * 512],
                                lhsT=ctxT[2 * p + qb // 8][:, (qb % 8) * P:(qb % 8 + 1) * P],
                                rhs=wo_sb[:, p, n * 512:(n + 1) * 512],
                                start=(p == 0), stop=False)

                def emit_b(qb):
                    oa = tiles.pop(qb)
                    for n in range(2):
                        nc.tensor.matmul(
                            oa[:, n * 512:(n + 1) * 512],
                            lhsT=ctxT[6 + qb // 8][:, (qb % 8) * P:(qb % 8 + 1) * P],
                            rhs=wo_sb[:, 3, n * 512:(n + 1) * 512],
                            start=False, stop=True)
                    ot = otp.tile([P, D], bf16, name="ot", tag="ot")
                    # cast on ACT (idle after the last exp) so PSUM slot
                    # recycling doesn't queue behind the tail normalize on DVE
                    nc.scalar.copy(ot, oa)
                    stq[qb % 2].dma_start(
                        out=out_d.ap()[qb * P:(qb + 1) * P, :], in_=ot)

                for qb in range(4):
                    emit_a(qb)
                for qb in range(16):
                    emit_b(qb)
                    if qb + 4 < 16:
                        emit_a(qb + 4)

    nc.finalize()
    return nc


def _prep_in_maps(q, k, v, Wq, bq, Wk, bk, Wv, bv, Wo, bo):
    in_maps = []
    for c in range(NCORES):
        b, g = c // 2, c % 2
        sl = slice(g * DL, (g + 1) * DL)
        bvl = np.asarray(bv)[sl].astype(np.float32)
        bvb = np.zeros(520, np.float32)
        for h in range(8):
            bvb[65 * h:65 * h + 64] = bvl[64 * h:64 * h + 64]
        bvb = np.broadcast_to(bvb, (P, 520))
        def tile_x(x):
            xt = np.ascontiguousarray(np.asarray(x)[b].T).astype(BF16)  # (1024, 2048)
            return xt.reshape(8, P, S)
        def tile_w(w):
            # (1024, DL) -> (P, 8, DL) partition-major
            return np.ascontiguousarray(
                np.asarray(w).reshape(8, P, -1).transpose(1, 0, 2))
        in_maps.append({
            "xq": tile_x(q),
            "xk": tile_x(k),
            "xv": tile_x(v),
            "wq": tile_w(np.asarray(Wq)[sl, :].T.astype(BF16)),
            "wk": tile_w(np.asarray(Wk)[sl, :].T.astype(BF16)),
            "wv": tile_w(np.asarray(Wv)[sl, :].T.astype(BF16)),
            "wo": np.ascontiguousarray(
                np.asarray(Wo)[:, sl].T.astype(BF16).reshape(4, P, D).transpose(1, 0, 2)),
            "bq": np.ascontiguousarray(np.asarray(bq)[sl].reshape(4, P).T).astype(np.float32),
            "bk": np.ascontiguousarray(np.asarray(bk)[sl].reshape(4, P).T).astype(np.float32),
            "bvb": np.ascontiguousarray(bvb).astype(BF16),
        })
    return in_maps


def _get_runner():
    """Build nc + jitted SPMD executor once; reuse across kernel() calls."""
    if "runner" in _CACHE:
        return _CACHE["runner"]
    import jax
    import jax.numpy as jnp
    from jax.sharding import Mesh, PartitionSpec
    from jax.experimental.shard_map import shard_map
    from concourse import mybir
    from concourse.bass2jax import (_bass_exec_p, install_neuronx_cc_hook,
                                    partition_id_tensor)

    nc = _build_nc()
    install_neuronx_cc_hook()

    partition_name = nc.partition_id_tensor.name if nc.partition_id_tensor else None
    in_names, out_names, out_avals, zero_shapes = [], [], [], []
    for alloc in nc.m.functions[0].allocations:
        if not isinstance(alloc, mybir.MemoryLocationSet):
            continue
        name = alloc.memorylocations[0].name
        if alloc.kind == "ExternalInput":
            if name != partition_name:
                in_names.append(name)
        elif alloc.kind == "ExternalOutput":
            shape = tuple(alloc.tensor_shape)
            dtype = mybir.dt.np(alloc.dtype)
            out_names.append(name)
            out_avals.append(jax.core.ShapedArray(shape, dtype))
            zero_shapes.append((shape, dtype))
    n_params = len(in_names)
    all_in_names = list(in_names) + list(out_names)
    if partition_name is not None:
        all_in_names.append(partition_name)

    def _body(*args):
        operands = list(args)
        if partition_name is not None:
            operands.append(partition_id_tensor())
        outs = _bass_exec_p.bind(
            *operands,
            out_avals=tuple(out_avals),
            in_names=tuple(all_in_names),
            out_names=tuple(out_names),
            lowering_input_output_aliases=(),
            sim_require_finite=True,
            sim_require_nnan=True,
            nc=nc,
        )
        return tuple(outs)

    devices = jax.devices()[:NCORES]
    mesh = Mesh(np.asarray(devices), ("core",))
    n_outs = len(out_names)
    sharded = jax.jit(
        shard_map(_body, mesh=mesh,
                  in_specs=(PartitionSpec("core"),) * (n_params + n_outs),
                  out_specs=(PartitionSpec("core"),) * n_outs,
                  check_rep=False),
        donate_argnums=tuple(range(n_params, n_params + n_outs)),
        keep_unused=True,
    )
    runner = dict(nc=nc, sharded=sharded, in_names=in_names,
                  out_names=out_names, zero_shapes=zero_shapes,
                  out_avals=out_avals)
    _CACHE["runner"] = runner
    return runner


def kernel(q, k, v, Wq, bq, Wk, bk, Wv, bv, Wo, bo):
    global LAST_RESULTS
    r = _get_runner()
    in_maps = _prep_in_maps(q, k, v, Wq, bq, Wk, bk, Wv, bv, Wo, bo)

    concat_in = [np.concatenate([m[name] for m in in_maps], axis=0)
                 for name in r["in_names"]]
    concat_zeros = [np.zeros((NCORES * s[0], *s[1:]), d)
                    for (s, d) in r["zero_shapes"]]
    out_arrs = r["sharded"](*concat_in, *concat_zeros)
    results = [
        {name: np.asarray(out_arrs[i]).reshape(NCORES, *r["out_avals"][i].shape)[c]
         for i, name in enumerate(r["out_names"])}
        for c in range(NCORES)
    ]
    LAST_RESULTS = results

    bo_f = np.asarray(bo).astype(np.float32)
    out = np.empty((B, S, D), np.float32)
    for b in range(B):
        out[b] = (results[2 * b]["out"].astype(np.float32)
                  + results[2 * b + 1]["out"].astype(np.float32)
                  + bo_f)
    return out

